# revision 45
# baseline (speedup 1.0000x reference)
"""Cross-attention (ragged graph pairs) Trainium2 Bass kernel.

Problem: B=64 graph pairs, N=512 max nodes, D=128 hidden.
  k = h @ Wk.T + bk ; q = h @ Wq.T + bq  (per graph, shared weights)
  o1 = softmax_mask(q1 k2^T * t, len2) @ k2, rows masked by len1
  o2 = softmax_mask(q2 k1^T * t, len1) @ k1, rows masked by len2

Math restructure (exact up to float rounding):
  s1[n,m] = q1[n]·k2[m] = h1[n]·M·h2[m] + u2[m] + v1[n] + c
  with M = Wk^T Wq, u2[m] = h2[m]·(Wk^T bq), v1[n] = h1[n]·(Wq^T bk),
  c = bk·bq.  exp(t(v1[n]+c)) multiplies numerator and denominator of the
  softmax identically => dropped.  exp(t·u2[m]) folds into the key mask
  ("emask", host-computed).  Values bias: softmax weights sum to 1 on valid
  rows, so o = a @ k_nobias + bk; the +bk happens on the host during gather.
  The projected tensors w = M^T @ h_short, v = M @ h_short are tiny host
  GEMMs (the big attention matmuls stay on device); scores then only need
  raw h of the longer graph as the other operand:
     s1T = w2T.T @ h1T = h2T.T @ v1T      (choose by which side is shorter)

Device work per direction:
  scores sT[m,n] (f32r matmuls), p = exp(t*s) (ACT, bf16 out),
  knat[m,d] = emask[m]*(h·Wk^T) via bf16 matmuls with emask as column 128,
  o[n,(d,den)] = sum_mt p_chunk.T @ knat (bf16 matmuls, f32 psum),
  PSUM->SBUF copy, one DMA per slot.  Output is unnormalized; the host
  divides by the denominator column during gather (rows >= len are sliced
  away, so no query-side masking is needed anywhere).

Sharding: batches are packed into 8 slots x 8 cores by a deterministic
annealing+hill-climb search minimizing padded tile work; every core runs
the identical SPMD program (slot loop bounds = max tile counts in the
slot); raggedness inside a slot is handled by mask data, not code.
"""
import sys
if "/opt/trn_rl_repo" not in sys.path:
    sys.path.insert(0, "/opt/trn_rl_repo")

import numpy as np
import ml_dtypes
import concourse.bacc as bacc
import concourse.tile as tile
from concourse import mybir
from concourse.bass_utils import run_bass_kernel_spmd

B, N, D = 64, 512, 128
NCORES = 8
NSLOTS = B // NCORES

F32 = mybir.dt.float32
F32R = mybir.dt.float32r
BF16 = mybir.dt.bfloat16

# consts layout (columns of the [128, CW] f32 const array)
_E1 = 0                     # emask from len1 (key-side, exp-scaled) [128,NSLOTS,4]
_E2 = _E1 + NSLOTS * 4      # emask from len2
CW = _E2 + NSLOTS * 4

_cache = {}


def _build(T1s, T2s, t_val, reps=1):
    """Build the SPMD program for slot tile-counts T1s/T2s."""
    L1s = [128 * x for x in T1s]
    L2s = [128 * x for x in T2s]
    LTs = [a + b for a, b in zip(L1s, L2s)]
    Lmx = [max(a, b) for a, b in zip(L1s, L2s)]
    Lmn = [min(a, b) for a, b in zip(L1s, L2s)]
    offh = np.concatenate([[0], np.cumsum(LTs)]).astype(int)
    offr = np.concatenate([[0], np.cumsum(Lmx)]).astype(int)
    offv = np.concatenate([[0], np.cumsum([2 * x for x in Lmn])]).astype(int)
    off12 = np.concatenate([[0], np.cumsum([a + b for a, b in zip(T1s, T2s)])]).astype(int)

    nc = bacc.Bacc("TRN2", target_bir_lowering=False, debug=False,
                   num_devices=NCORES)
    consts_d = nc.dram_tensor("consts", [128, CW], F32R, kind="ExternalInput")
    constsb_d = nc.dram_tensor("constsb", [128, 128], BF16, kind="ExternalInput")
    hTr_d = nc.dram_tensor("hTr", [128, int(offr[-1])], F32R, kind="ExternalInput")
    wv_d = nc.dram_tensor("wv", [128, int(offv[-1])], F32R, kind="ExternalInput")
    hTb_d = nc.dram_tensor("hTb", [128, int(offh[-1])], BF16, kind="ExternalInput")
    o12_d = nc.dram_tensor("o12", [128, int(off12[-1]), D + 1], F32,
                           kind="ExternalOutput")

    with tile.TileContext(nc, pool_alloc_mode="queue") as tc:
        from contextlib import ExitStack
        with ExitStack() as ctx:
            constp = ctx.enter_context(tc.tile_pool(name="constp", bufs=1))
            hp = ctx.enter_context(tc.tile_pool(name="hp", bufs=4))
            hbp = ctx.enter_context(tc.tile_pool(name="hbp", bufs=4))
            wp = ctx.enter_context(tc.tile_pool(name="wp", bufs=4))
            knp = ctx.enter_context(tc.tile_pool(name="knp", bufs=6))
            pp = ctx.enter_context(tc.tile_pool(name="pp", bufs=8))
            outp = ctx.enter_context(tc.tile_pool(name="outp", bufs=6))
            sps_pool = ctx.enter_context(tc.tile_pool(name="sps", bufs=2, space="PSUM"))
            knps = ctx.enter_context(tc.tile_pool(name="knps", bufs=1, space="PSUM"))
            ops_pool = ctx.enter_context(tc.tile_pool(name="ops", bufs=3, space="PSUM"))

            C = constp.tile([128, CW], F32R)
            nc.sync.dma_start(out=C, in_=consts_d[:, :])
            Cb = constp.tile([128, 128], BF16)
            nc.gpsimd.dma_start(out=Cb, in_=constsb_d[:, :])
            WkTb = Cb[:, 0:128]
            e1m = C[:, _E1:_E1 + NSLOTS * 4].bitcast(F32).rearrange(
                "p (j t) -> p j t", j=NSLOTS)
            e2m = C[:, _E2:_E2 + NSLOTS * 4].bitcast(F32).rearrange(
                "p (j t) -> p j t", j=NSLOTS)

            # PE warm-up: ~3.4us of back-to-back matmuls during the startup
            # head releases the HAM clock gate (1.2 -> 2.4 GHz) before the
            # first real scores matmul; results are discarded.
            warm = sps_pool.tile([128, 2, 512], F32, tag="spair")
            for _wi in range(30):
                nc.tensor.matmul(warm[:, 0, 0:128], WkTb, WkTb,
                                 start=True, stop=True)

            for _rep in range(reps):
                for j in range(NSLOTS):
                    T1, T2 = T1s[j], T2s[j]
                    L1, L2 = L1s[j], L2s[j]
                    LT = L1 + L2
                    P2 = L2 <= L1
                    LR, LP = Lmx[j], Lmn[j]

                    hTr = hp.tile([128, LR], F32R, tag="hTr")
                    nc.sync.dma_start(out=hTr,
                                      in_=hTr_d[:, int(offr[j]):int(offr[j]) + LR])
                    vw = wp.tile([128, 2, LP], F32R, tag="vw")
                    nc.sync.dma_start(
                        out=vw, in_=wv_d[:, int(offv[j]):int(offv[j]) + 2 * LP])
                    hTb = hbp.tile([128, LT], BF16, tag="hTb")
                    nc.gpsimd.dma_start(out=hTb,
                                        in_=hTb_d[:, int(offh[j]):int(offh[j]) + LT])

                    # scores lhsT/rhs depend on which graph was projected
                    if P2:
                        d1_lhs, d1_loff = vw[:, 0, :], 0          # w2T chunks
                        d1_rhs = hTr[:, 0:L1]                     # raw h1T
                        d2_lhs, d2_loff = hTr, 0                  # raw h1T
                        d2_rhs = vw[:, 1, 0:L2]                   # v2T
                    else:
                        d1_lhs, d1_loff = hTr, 0                  # raw h2T
                        d1_rhs = vw[:, 1, 0:L1]                   # v1T
                        d2_lhs, d2_loff = vw[:, 0, :], 0          # w1T chunks
                        d2_rhs = hTr[:, 0:L2]                     # raw h2T
                    dirs = (
                        (T1, T2, d1_lhs, d1_loff, d1_rhs, 1, 0),
                        (T2, T1, d2_lhs, d2_loff, d2_rhs, 0, T1),
                    )
                    # pass 1: scores (sT layout [m, n]) + exp, in m-tile pairs
                    all_ptiles = []
                    for (Tn, Tm, s_lhs, s_loff, s_rhs, kni, obase) in dirs:
                        Ln = 128 * Tn
                        ptiles = []
                        for mp0 in range(0, Tm, 2):
                            mps = min(2, Tm - mp0)
                            sps = sps_pool.tile([128, mps, 512], F32, tag="spair")
                            for k in range(mps):
                                mt = mp0 + k
                                nc.tensor.matmul(
                                    sps[:, k, 0:Ln],
                                    s_lhs[:, s_loff + 128 * mt:s_loff + 128 * (mt + 1)],
                                    s_rhs,
                                    start=True, stop=True)
                            pt = pp.tile([128, mps, Ln], BF16, tag="p")
                            nc.scalar.activation(
                                out=pt, in_=sps[:, 0:mps, 0:Ln],
                                func=mybir.ActivationFunctionType.Exp,
                                scale=float(t_val))
                            ptiles.append(pt)
                        all_ptiles.append(ptiles)

                    # natural-layout keys scaled by emask + emask column
                    knats = []
                    for (Tm, koff, e_ap) in ((T1, 0, e1m), (T2, L1, e2m)):
                        kn = knp.tile([128, Tm, 129], BF16, tag="knat")
                        kps = knps.tile([128, Tm, 128], F32, tag="knps")
                        for mt in range(Tm):
                            nc.tensor.matmul(
                                kps[:, mt, :],
                                hTb[:, koff + 128 * mt:koff + 128 * (mt + 1)],
                                WkTb, start=True, stop=True)
                        nc.vector.tensor_mul(
                            kn[:, 0:Tm, 0:128], kps,
                            e_ap[:, j, 0:Tm].broadcast_to([128, Tm, 128]))
                        nc.vector.tensor_copy(kn[:, :, 128], e_ap[:, j, 0:Tm])
                        knats.append(kn)

                    # pass 2: output accumulation in n-tile pairs; both
                    # directions land in one tile -> one DMA per slot.
                    # Output stays UNNORMALIZED; the denominator rides in
                    # column 128 and the host divides during gather.
                    osb = outp.tile([128, T1 + T2, 129], F32, tag="osb")
                    for di, (Tn, Tm, s_lhs, s_loff, s_rhs, kni, obase) in enumerate(dirs):
                        ptiles = all_ptiles[di]
                        kn = knats[kni]
                        for np0 in range(0, Tn, 2):
                            nps = min(2, Tn - np0)
                            ops = ops_pool.tile([128, nps, 129], F32, tag="opair")
                            for k in range(nps):
                                nt = np0 + k
                                for mt in range(Tm):
                                    nc.tensor.matmul(
                                        ops[:, k, :],
                                        ptiles[mt // 2][:, mt % 2,
                                                        128 * nt:128 * (nt + 1)],
                                        kn[:, mt, :],
                                        start=(mt == 0), stop=(mt == Tm - 1))
                            dst = osb[:, obase + np0:obase + np0 + nps, :]
                            nc.vector.tensor_copy(dst, ops[:, 0:nps, :])
                    ro = int(off12[j])
                    nc.sync.dma_start(out=o12_d[:, ro:ro + T1 + T2, :],
                                      in_=osb[:, 0:T1 + T2, :])

    nc.compile()
    return nc


_plan_cache = {}


def _plan(len1, len2):
    """Assign batches to slots minimizing padded work; deterministic."""
    pk = (np.asarray(len1).tobytes(), np.asarray(len2).tobytes())
    if pk in _plan_cache:
        return _plan_cache[pk]
    t1 = np.ceil(np.asarray(len1) / 128).astype(int)
    t2 = np.ceil(np.asarray(len2) / 128).astype(int)
    order = np.array(np.lexsort((-t2, -t1)))  # descending (t1, t2)
    slots = [list(order[j * NCORES:(j + 1) * NCORES]) for j in range(NSLOTS)]

    def slot_cost(members):
        m1 = max(int(t1[b]) for b in members)
        m2 = max(int(t2[b]) for b in members)
        return 2.0 * m1 * m2 + 1.0 * (m1 + m2)

    tt1 = [int(x) for x in t1]
    tt2 = [int(x) for x in t2]

    def scost(s):
        m1 = max(tt1[b] for b in s)
        m2 = max(tt2[b] for b in s)
        return 2 * m1 * m2 + m1 + m2

    rng = np.random.RandomState(0)
    best = (sum(scost(s) for s in slots), [list(s) for s in slots])
    for _restart in range(10):
        perm = list(rng.permutation(len(t1)))
        cand = [perm[j * NCORES:(j + 1) * NCORES] for j in range(NSLOTS)]
        T = 4.0
        for _it in range(20000):
            ja = rng.randint(NSLOTS); jb = rng.randint(NSLOTS)
            if ja == jb:
                continue
            ia = rng.randint(NCORES); ib = rng.randint(NCORES)
            before = scost(cand[ja]) + scost(cand[jb])
            cand[ja][ia], cand[jb][ib] = cand[jb][ib], cand[ja][ia]
            after = scost(cand[ja]) + scost(cand[jb])
            if after > before and rng.rand() >= np.exp(-(after - before) / max(T, 1e-3)):
                cand[ja][ia], cand[jb][ib] = cand[jb][ib], cand[ja][ia]
            T *= 0.9997
        c = sum(scost(s) for s in cand)
        if c < best[0]:
            best = (c, [list(s) for s in cand])
    slots = best[1]

    improved = True
    rounds = 0
    while improved and rounds < 20:
        improved = False
        rounds += 1
        for ja in range(NSLOTS):
            for jb in range(ja + 1, NSLOTS):
                base = slot_cost(slots[ja]) + slot_cost(slots[jb])
                bsw = None
                for ia in range(NCORES):
                    for ib in range(NCORES):
                        sa = slots[ja][:]
                        sb = slots[jb][:]
                        sa[ia], sb[ib] = sb[ib], sa[ia]
                        c = slot_cost(sa) + slot_cost(sb)
                        if c < base - 1e-9 and (bsw is None or c < bsw[0]):
                            bsw = (c, ia, ib)
                if bsw is not None:
                    _, ia, ib = bsw
                    slots[ja][ia], slots[jb][ib] = slots[jb][ib], slots[ja][ia]
                    improved = True

    slots.sort(key=slot_cost, reverse=True)
    order = np.array([b for s in slots for b in s])
    T1s, T2s = [], []
    for j in range(NSLOTS):
        members = order[j * NCORES:(j + 1) * NCORES]
        T1s.append(int(t1[members].max()))
        T2s.append(int(t2[members].max()))
    out = (order, tuple(T1s), tuple(T2s))
    _plan_cache[pk] = out
    return out


def kernel(h1, h2, Wk, bk, Wq, bq, t, len1, len2, _reps=1, _return_raw=False,
           _trace=False):
    h1 = np.asarray(h1, dtype=np.float32)
    h2 = np.asarray(h2, dtype=np.float32)
    Wk = np.asarray(Wk, np.float32)
    Wq = np.asarray(Wq, np.float32)
    bk = np.asarray(bk, np.float32)
    bq = np.asarray(bq, np.float32)
    len1 = np.asarray(len1).astype(np.int64)
    len2 = np.asarray(len2).astype(np.int64)
    t_val = float(np.asarray(t))

    order, T1s, T2s = _plan(len1, len2)
    L1s = [128 * x for x in T1s]
    L2s = [128 * x for x in T2s]
    LTs = [a + b for a, b in zip(L1s, L2s)]
    Lmx = [max(a, b) for a, b in zip(L1s, L2s)]
    Lmn = [min(a, b) for a, b in zip(L1s, L2s)]
    offh = np.concatenate([[0], np.cumsum(LTs)]).astype(int)
    offr = np.concatenate([[0], np.cumsum(Lmx)]).astype(int)
    offv = np.concatenate([[0], np.cumsum([2 * x for x in Lmn])]).astype(int)
    off12 = np.concatenate([[0], np.cumsum([a + b for a, b in zip(T1s, T2s)])]).astype(int)

    key = (T1s, T2s, t_val, _reps)
    if key not in _cache:
        _cache[key] = _build(T1s, T2s, t_val, reps=_reps)
    nc = _cache[key]

    h1T = np.ascontiguousarray(h1.transpose(0, 2, 1))  # [B, D, N]
    h2T = np.ascontiguousarray(h2.transpose(0, 2, 1))
    M = Wk.T @ Wq
    g_u = Wk.T @ bq                      # key-side bias direction
    u1 = h1 @ g_u                        # [B, N]
    u2 = h2 @ g_u
    pos = np.arange(N)

    in_maps = []
    for c in range(NCORES):
        consts = np.zeros((128, CW), dtype=np.float32)
        constsb = np.zeros((128, 128), dtype=ml_dtypes.bfloat16)
        constsb[:, 0:128] = Wk.T.astype(ml_dtypes.bfloat16)
        hTb_c = np.zeros((128, offh[-1]), dtype=np.float32)
        hTr_c = np.zeros((128, offr[-1]), dtype=np.float32)
        wv_c = np.zeros((128, offv[-1]), dtype=np.float32)
        for j in range(NSLOTS):
            b = int(order[j * NCORES + c])
            ho = offh[j]
            hTb_c[:, ho:ho + L1s[j]] = h1T[b, :, :L1s[j]]
            hTb_c[:, ho + L1s[j]:ho + LTs[j]] = h2T[b, :, :L2s[j]]
            P2 = L2s[j] <= L1s[j]
            hR = h1T[b, :, :Lmx[j]] if P2 else h2T[b, :, :Lmx[j]]
            hP = h2T[b, :, :Lmn[j]] if P2 else h1T[b, :, :Lmn[j]]
            hTr_c[:, offr[j]:offr[j] + Lmx[j]] = hR
            wv_c[:, offv[j]:offv[j] + Lmn[j]] = M.T @ hP
            wv_c[:, offv[j] + Lmn[j]:offv[j] + 2 * Lmn[j]] = M @ hP
            m1 = (pos[:512] < len1[b]).astype(np.float32)
            m2 = (pos[:512] < len2[b]).astype(np.float32)
            e1 = m1 * np.exp(t_val * u1[b, :512].astype(np.float64)).astype(np.float32)
            e2 = m2 * np.exp(t_val * u2[b, :512].astype(np.float64)).astype(np.float32)
            consts[:, _E1 + j * 4:_E1 + (j + 1) * 4] = e1.reshape(4, 128).T
            consts[:, _E2 + j * 4:_E2 + (j + 1) * 4] = e2.reshape(4, 128).T
        in_maps.append({
            "consts": consts, "constsb": constsb,
            "hTr": hTr_c, "wv": wv_c,
            "hTb": hTb_c.astype(ml_dtypes.bfloat16),
        })

    res = run_bass_kernel_spmd(nc, in_maps, list(range(NCORES)), trace=_trace)
    if _return_raw:
        return res

    o1 = np.zeros((B, N, D), dtype=np.float32)
    o2 = np.zeros((B, N, D), dtype=np.float32)
    for c in range(NCORES):
        r = res.results[c]
        for j in range(NSLOTS):
            b = int(order[j * NCORES + c])
            n1, n2 = int(len1[b]), int(len2[b])
            seg1 = r["o12"][:, off12[j]:off12[j] + T1s[j], :]       # [128,T1,129]
            seg2 = r["o12"][:, off12[j] + T1s[j]:off12[j] + T1s[j] + T2s[j], :]
            seg1 = seg1.transpose(1, 0, 2).reshape(-1, D + 1)[:n1]
            seg2 = seg2.transpose(1, 0, 2).reshape(-1, D + 1)[:n2]
            o1[b, :n1, :] = seg1[:, :D] / seg1[:, D:] + bk
            o2[b, :n2, :] = seg2[:, :D] / seg2[:, D:] + bk
    return o1, o2


# revision 46
# speedup vs baseline: 1.1025x; 1.1025x over previous
"""Cross-attention (ragged graph pairs) Trainium2 Bass kernel.

Problem: B=64 graph pairs, N=512 max nodes, D=128 hidden.
  k = h @ Wk.T + bk ; q = h @ Wq.T + bq  (per graph, shared weights)
  o1 = softmax_mask(q1 k2^T * t, len2) @ k2, rows masked by len1
  o2 = softmax_mask(q2 k1^T * t, len1) @ k1, rows masked by len2

Math restructure (exact up to float rounding):
  s1[n,m] = q1[n]·k2[m] = h1[n]·M·h2[m] + u2[m] + v1[n] + c
  with M = Wk^T Wq, u2[m] = h2[m]·(Wk^T bq), v1[n] = h1[n]·(Wq^T bk),
  c = bk·bq.  exp(t(v1[n]+c)) multiplies numerator and denominator of the
  softmax identically => dropped.  exp(t·u2[m]) folds into the key mask
  ("emask", host-computed).  Values bias: softmax weights sum to 1 on valid
  rows, so o = a @ k_nobias + bk; the +bk happens on the host during gather.
  The projected tensors w = M^T @ h_short, v = M @ h_short are tiny host
  GEMMs (the big attention matmuls stay on device); scores then only need
  raw h of the longer graph as the other operand:
     s1T = w2T.T @ h1T = h2T.T @ v1T      (choose by which side is shorter)

Device work per direction:
  scores sT[m,n] (f32r matmuls), p = exp(t*s) (ACT, bf16 out),
  knat[m,d] = emask[m]*(h·Wk^T) via bf16 matmuls with emask as column 128,
  o[n,(d,den)] = sum_mt p_chunk.T @ knat (bf16 matmuls, f32 psum),
  PSUM->SBUF copy, one DMA per slot.  Output is unnormalized; the host
  divides by the denominator column during gather (rows >= len are sliced
  away, so no query-side masking is needed anywhere).

Sharding: batches are packed into 8 slots x 8 cores by a deterministic
annealing+hill-climb search minimizing padded tile work; every core runs
the identical SPMD program (slot loop bounds = max tile counts in the
slot); raggedness inside a slot is handled by mask data, not code.
"""
import sys
if "/opt/trn_rl_repo" not in sys.path:
    sys.path.insert(0, "/opt/trn_rl_repo")

import numpy as np
import ml_dtypes
import concourse.bacc as bacc
import concourse.tile as tile
from concourse import mybir
from concourse.bass_utils import run_bass_kernel_spmd

B, N, D = 64, 512, 128
NCORES = 8
NSLOTS = B // NCORES

F32 = mybir.dt.float32
F32R = mybir.dt.float32r
BF16 = mybir.dt.bfloat16

# consts layout (columns of the [128, CW] f32 const array)
_E1 = 0                     # emask from len1 (key-side, exp-scaled) [128,NSLOTS,4]
_E2 = _E1 + NSLOTS * 4      # emask from len2
CW = _E2 + NSLOTS * 4

_cache = {}


def _build(T1s, T2s, t_val, reps=1):
    """Build the SPMD program for slot tile-counts T1s/T2s."""
    L1s = [128 * x for x in T1s]
    L2s = [128 * x for x in T2s]
    LTs = [a + b for a, b in zip(L1s, L2s)]
    Lmx = [max(a, b) for a, b in zip(L1s, L2s)]
    Lmn = [min(a, b) for a, b in zip(L1s, L2s)]
    offh = np.concatenate([[0], np.cumsum(LTs)]).astype(int)
    offr = np.concatenate([[0], np.cumsum(Lmx)]).astype(int)
    offv = np.concatenate([[0], np.cumsum([2 * x for x in Lmn])]).astype(int)
    off12 = np.concatenate([[0], np.cumsum([a + b for a, b in zip(T1s, T2s)])]).astype(int)

    nc = bacc.Bacc("TRN2", target_bir_lowering=False, debug=False,
                   num_devices=NCORES)
    consts_d = nc.dram_tensor("consts", [128, CW], F32R, kind="ExternalInput")
    constsb_d = nc.dram_tensor("constsb", [128, 128], BF16, kind="ExternalInput")
    hTr_d = nc.dram_tensor("hTr", [128, int(offr[-1])], F32R, kind="ExternalInput")
    wv_d = nc.dram_tensor("wv", [128, int(offv[-1])], F32R, kind="ExternalInput")
    hTb_d = nc.dram_tensor("hTb", [128, int(offh[-1])], BF16, kind="ExternalInput")
    o12_d = nc.dram_tensor("o12", [128, int(off12[-1]), D + 1], F32,
                           kind="ExternalOutput")

    with tile.TileContext(nc, pool_alloc_mode="queue") as tc:
        from contextlib import ExitStack
        with ExitStack() as ctx:
            constp = ctx.enter_context(tc.tile_pool(name="constp", bufs=1))
            hp = ctx.enter_context(tc.tile_pool(name="hp", bufs=4))
            hbp = ctx.enter_context(tc.tile_pool(name="hbp", bufs=4))
            wp = ctx.enter_context(tc.tile_pool(name="wp", bufs=4))
            knp = ctx.enter_context(tc.tile_pool(name="knp", bufs=6))
            pp = ctx.enter_context(tc.tile_pool(name="pp", bufs=8))
            outp = ctx.enter_context(tc.tile_pool(name="outp", bufs=6))
            sps_pool = ctx.enter_context(tc.tile_pool(name="sps", bufs=2, space="PSUM"))
            knps = ctx.enter_context(tc.tile_pool(name="knps", bufs=1, space="PSUM"))
            ops_pool = ctx.enter_context(tc.tile_pool(name="ops", bufs=3, space="PSUM"))

            C = constp.tile([128, CW], F32R)
            nc.sync.dma_start(out=C, in_=consts_d[:, :])
            Cb = constp.tile([128, 128], BF16)
            nc.gpsimd.dma_start(out=Cb, in_=constsb_d[:, :])
            WkTb = Cb[:, 0:128]
            e1m = C[:, _E1:_E1 + NSLOTS * 4].bitcast(F32).rearrange(
                "p (j t) -> p j t", j=NSLOTS)
            e2m = C[:, _E2:_E2 + NSLOTS * 4].bitcast(F32).rearrange(
                "p (j t) -> p j t", j=NSLOTS)

            for _rep in range(reps):
                for j in range(NSLOTS):
                    T1, T2 = T1s[j], T2s[j]
                    L1, L2 = L1s[j], L2s[j]
                    LT = L1 + L2
                    P2 = L2 <= L1
                    LR, LP = Lmx[j], Lmn[j]

                    hTr = hp.tile([128, LR], F32R, tag="hTr")
                    nc.sync.dma_start(out=hTr,
                                      in_=hTr_d[:, int(offr[j]):int(offr[j]) + LR])
                    vw = wp.tile([128, 2, LP], F32R, tag="vw")
                    nc.sync.dma_start(
                        out=vw, in_=wv_d[:, int(offv[j]):int(offv[j]) + 2 * LP])
                    hTb = hbp.tile([128, LT], BF16, tag="hTb")
                    nc.gpsimd.dma_start(out=hTb,
                                        in_=hTb_d[:, int(offh[j]):int(offh[j]) + LT])

                    # scores lhsT/rhs depend on which graph was projected
                    if P2:
                        d1_lhs, d1_loff = vw[:, 0, :], 0          # w2T chunks
                        d1_rhs = hTr[:, 0:L1]                     # raw h1T
                        d2_lhs, d2_loff = hTr, 0                  # raw h1T
                        d2_rhs = vw[:, 1, 0:L2]                   # v2T
                    else:
                        d1_lhs, d1_loff = hTr, 0                  # raw h2T
                        d1_rhs = vw[:, 1, 0:L1]                   # v1T
                        d2_lhs, d2_loff = vw[:, 0, :], 0          # w1T chunks
                        d2_rhs = hTr[:, 0:L2]                     # raw h2T
                    dirs = (
                        (T1, T2, d1_lhs, d1_loff, d1_rhs, 1, 0),
                        (T2, T1, d2_lhs, d2_loff, d2_rhs, 0, T1),
                    )
                    # pass 1: scores (sT layout [m, n]) + exp, in m-tile pairs
                    all_ptiles = []
                    for (Tn, Tm, s_lhs, s_loff, s_rhs, kni, obase) in dirs:
                        Ln = 128 * Tn
                        ptiles = []
                        for mp0 in range(0, Tm, 2):
                            mps = min(2, Tm - mp0)
                            sps = sps_pool.tile([128, mps, 512], F32, tag="spair")
                            for k in range(mps):
                                mt = mp0 + k
                                nc.tensor.matmul(
                                    sps[:, k, 0:Ln],
                                    s_lhs[:, s_loff + 128 * mt:s_loff + 128 * (mt + 1)],
                                    s_rhs,
                                    start=True, stop=True)
                            pt = pp.tile([128, mps, Ln], BF16, tag="p")
                            nc.scalar.activation(
                                out=pt, in_=sps[:, 0:mps, 0:Ln],
                                func=mybir.ActivationFunctionType.Exp,
                                scale=float(t_val))
                            ptiles.append(pt)
                        all_ptiles.append(ptiles)

                    # natural-layout keys scaled by emask + emask column
                    knats = []
                    for (Tm, koff, e_ap) in ((T1, 0, e1m), (T2, L1, e2m)):
                        kn = knp.tile([128, Tm, 129], BF16, tag="knat")
                        kps = knps.tile([128, Tm, 128], F32, tag="knps")
                        for mt in range(Tm):
                            nc.tensor.matmul(
                                kps[:, mt, :],
                                hTb[:, koff + 128 * mt:koff + 128 * (mt + 1)],
                                WkTb, start=True, stop=True)
                        nc.vector.tensor_mul(
                            kn[:, 0:Tm, 0:128], kps,
                            e_ap[:, j, 0:Tm].broadcast_to([128, Tm, 128]))
                        nc.vector.tensor_copy(kn[:, :, 128], e_ap[:, j, 0:Tm])
                        knats.append(kn)

                    # pass 2: output accumulation in n-tile pairs; both
                    # directions land in one tile -> one DMA per slot.
                    # Output stays UNNORMALIZED; the denominator rides in
                    # column 128 and the host divides during gather.
                    osb = outp.tile([128, T1 + T2, 129], F32, tag="osb")
                    for di, (Tn, Tm, s_lhs, s_loff, s_rhs, kni, obase) in enumerate(dirs):
                        ptiles = all_ptiles[di]
                        kn = knats[kni]
                        for np0 in range(0, Tn, 2):
                            nps = min(2, Tn - np0)
                            ops = ops_pool.tile([128, nps, 129], F32, tag="opair")
                            for k in range(nps):
                                nt = np0 + k
                                for mt in range(Tm):
                                    nc.tensor.matmul(
                                        ops[:, k, :],
                                        ptiles[mt // 2][:, mt % 2,
                                                        128 * nt:128 * (nt + 1)],
                                        kn[:, mt, :],
                                        start=(mt == 0), stop=(mt == Tm - 1))
                            dst = osb[:, obase + np0:obase + np0 + nps, :]
                            nc.vector.tensor_copy(dst, ops[:, 0:nps, :])
                    ro = int(off12[j])
                    nc.sync.dma_start(out=o12_d[:, ro:ro + T1 + T2, :],
                                      in_=osb[:, 0:T1 + T2, :])

    nc.compile()
    return nc


_plan_cache = {}


def _plan(len1, len2):
    """Assign batches to slots minimizing padded work; deterministic."""
    pk = (np.asarray(len1).tobytes(), np.asarray(len2).tobytes())
    if pk in _plan_cache:
        return _plan_cache[pk]
    t1 = np.ceil(np.asarray(len1) / 128).astype(int)
    t2 = np.ceil(np.asarray(len2) / 128).astype(int)
    order = np.array(np.lexsort((-t2, -t1)))  # descending (t1, t2)
    slots = [list(order[j * NCORES:(j + 1) * NCORES]) for j in range(NSLOTS)]

    def slot_cost(members):
        m1 = max(int(t1[b]) for b in members)
        m2 = max(int(t2[b]) for b in members)
        return 2.0 * m1 * m2 + 1.0 * (m1 + m2)

    tt1 = [int(x) for x in t1]
    tt2 = [int(x) for x in t2]

    def scost(s):
        m1 = max(tt1[b] for b in s)
        m2 = max(tt2[b] for b in s)
        return 2 * m1 * m2 + m1 + m2

    rng = np.random.RandomState(0)
    best = (sum(scost(s) for s in slots), [list(s) for s in slots])
    for _restart in range(10):
        perm = list(rng.permutation(len(t1)))
        cand = [perm[j * NCORES:(j + 1) * NCORES] for j in range(NSLOTS)]
        T = 4.0
        for _it in range(20000):
            ja = rng.randint(NSLOTS); jb = rng.randint(NSLOTS)
            if ja == jb:
                continue
            ia = rng.randint(NCORES); ib = rng.randint(NCORES)
            before = scost(cand[ja]) + scost(cand[jb])
            cand[ja][ia], cand[jb][ib] = cand[jb][ib], cand[ja][ia]
            after = scost(cand[ja]) + scost(cand[jb])
            if after > before and rng.rand() >= np.exp(-(after - before) / max(T, 1e-3)):
                cand[ja][ia], cand[jb][ib] = cand[jb][ib], cand[ja][ia]
            T *= 0.9997
        c = sum(scost(s) for s in cand)
        if c < best[0]:
            best = (c, [list(s) for s in cand])
    slots = best[1]

    improved = True
    rounds = 0
    while improved and rounds < 20:
        improved = False
        rounds += 1
        for ja in range(NSLOTS):
            for jb in range(ja + 1, NSLOTS):
                base = slot_cost(slots[ja]) + slot_cost(slots[jb])
                bsw = None
                for ia in range(NCORES):
                    for ib in range(NCORES):
                        sa = slots[ja][:]
                        sb = slots[jb][:]
                        sa[ia], sb[ib] = sb[ib], sa[ia]
                        c = slot_cost(sa) + slot_cost(sb)
                        if c < base - 1e-9 and (bsw is None or c < bsw[0]):
                            bsw = (c, ia, ib)
                if bsw is not None:
                    _, ia, ib = bsw
                    slots[ja][ia], slots[jb][ib] = slots[jb][ib], slots[ja][ia]
                    improved = True

    slots.sort(key=slot_cost, reverse=True)
    order = np.array([b for s in slots for b in s])
    T1s, T2s = [], []
    for j in range(NSLOTS):
        members = order[j * NCORES:(j + 1) * NCORES]
        T1s.append(int(t1[members].max()))
        T2s.append(int(t2[members].max()))
    out = (order, tuple(T1s), tuple(T2s))
    _plan_cache[pk] = out
    return out


def kernel(h1, h2, Wk, bk, Wq, bq, t, len1, len2, _reps=1, _return_raw=False,
           _trace=False):
    h1 = np.asarray(h1, dtype=np.float32)
    h2 = np.asarray(h2, dtype=np.float32)
    Wk = np.asarray(Wk, np.float32)
    Wq = np.asarray(Wq, np.float32)
    bk = np.asarray(bk, np.float32)
    bq = np.asarray(bq, np.float32)
    len1 = np.asarray(len1).astype(np.int64)
    len2 = np.asarray(len2).astype(np.int64)
    t_val = float(np.asarray(t))

    order, T1s, T2s = _plan(len1, len2)
    L1s = [128 * x for x in T1s]
    L2s = [128 * x for x in T2s]
    LTs = [a + b for a, b in zip(L1s, L2s)]
    Lmx = [max(a, b) for a, b in zip(L1s, L2s)]
    Lmn = [min(a, b) for a, b in zip(L1s, L2s)]
    offh = np.concatenate([[0], np.cumsum(LTs)]).astype(int)
    offr = np.concatenate([[0], np.cumsum(Lmx)]).astype(int)
    offv = np.concatenate([[0], np.cumsum([2 * x for x in Lmn])]).astype(int)
    off12 = np.concatenate([[0], np.cumsum([a + b for a, b in zip(T1s, T2s)])]).astype(int)

    key = (T1s, T2s, t_val, _reps)
    if key not in _cache:
        _cache[key] = _build(T1s, T2s, t_val, reps=_reps)
    nc = _cache[key]

    h1T = np.ascontiguousarray(h1.transpose(0, 2, 1))  # [B, D, N]
    h2T = np.ascontiguousarray(h2.transpose(0, 2, 1))
    M = Wk.T @ Wq
    g_u = Wk.T @ bq                      # key-side bias direction
    u1 = h1 @ g_u                        # [B, N]
    u2 = h2 @ g_u
    pos = np.arange(N)

    in_maps = []
    for c in range(NCORES):
        consts = np.zeros((128, CW), dtype=np.float32)
        constsb = np.zeros((128, 128), dtype=ml_dtypes.bfloat16)
        constsb[:, 0:128] = Wk.T.astype(ml_dtypes.bfloat16)
        hTb_c = np.zeros((128, offh[-1]), dtype=np.float32)
        hTr_c = np.zeros((128, offr[-1]), dtype=np.float32)
        wv_c = np.zeros((128, offv[-1]), dtype=np.float32)
        for j in range(NSLOTS):
            b = int(order[j * NCORES + c])
            ho = offh[j]
            hTb_c[:, ho:ho + L1s[j]] = h1T[b, :, :L1s[j]]
            hTb_c[:, ho + L1s[j]:ho + LTs[j]] = h2T[b, :, :L2s[j]]
            P2 = L2s[j] <= L1s[j]
            hR = h1T[b, :, :Lmx[j]] if P2 else h2T[b, :, :Lmx[j]]
            hP = h2T[b, :, :Lmn[j]] if P2 else h1T[b, :, :Lmn[j]]
            hTr_c[:, offr[j]:offr[j] + Lmx[j]] = hR
            wv_c[:, offv[j]:offv[j] + Lmn[j]] = M.T @ hP
            wv_c[:, offv[j] + Lmn[j]:offv[j] + 2 * Lmn[j]] = M @ hP
            m1 = (pos[:512] < len1[b]).astype(np.float32)
            m2 = (pos[:512] < len2[b]).astype(np.float32)
            e1 = m1 * np.exp(t_val * u1[b, :512].astype(np.float64)).astype(np.float32)
            e2 = m2 * np.exp(t_val * u2[b, :512].astype(np.float64)).astype(np.float32)
            consts[:, _E1 + j * 4:_E1 + (j + 1) * 4] = e1.reshape(4, 128).T
            consts[:, _E2 + j * 4:_E2 + (j + 1) * 4] = e2.reshape(4, 128).T
        in_maps.append({
            "consts": consts, "constsb": constsb,
            "hTr": hTr_c, "wv": wv_c,
            "hTb": hTb_c.astype(ml_dtypes.bfloat16),
        })

    res = run_bass_kernel_spmd(nc, in_maps, list(range(NCORES)), trace=_trace)
    if _return_raw:
        return res

    o1 = np.zeros((B, N, D), dtype=np.float32)
    o2 = np.zeros((B, N, D), dtype=np.float32)
    for c in range(NCORES):
        r = res.results[c]
        for j in range(NSLOTS):
            b = int(order[j * NCORES + c])
            n1, n2 = int(len1[b]), int(len2[b])
            seg1 = r["o12"][:, off12[j]:off12[j] + T1s[j], :]       # [128,T1,129]
            seg2 = r["o12"][:, off12[j] + T1s[j]:off12[j] + T1s[j] + T2s[j], :]
            seg1 = seg1.transpose(1, 0, 2).reshape(-1, D + 1)[:n1]
            seg2 = seg2.transpose(1, 0, 2).reshape(-1, D + 1)[:n2]
            o1[b, :n1, :] = seg1[:, :D] / seg1[:, D:] + bk
            o2[b, :n2, :] = seg2[:, :D] / seg2[:, D:] + bk
    return o1, o2
